# revision 25
# baseline (speedup 1.0000x reference)
"""Trainium2 Bass kernel for nn_LocalContextProcessor (local linear attention).

Computation (per 256-token window, fully independent):
    qkv = x @ W_qkv ; q,k,v split ; per head: q,k <- elu(.)+1
    ctx = k^T @ v ; attn = (q @ ctx) ; out = attn @ W_out + b_out

Sharding: data-parallel over the 64 windows (4 batch x 16 windows);
each of the 8 cores processes 8 consecutive windows (2048 tokens).
Weights are replicated to every core.

All matmuls run in bf16 (f32 PSUM accumulate): on TRN2 the PE runs bf16 at
1 cyc/row for any moving size, so every matmul is at the 16384 MAC/cycle
peak and the total PE work is the algorithmic floor (~557k cycles/core).
x is cast to bf16 and pre-transposed on the host into the (d-on-partition)
layout the PE contracts over, removing all on-chip transposes.

Per-core dataflow (all matmuls contract over the partition dim):
    q_T (j,n)  = [W_qkv chunk as lhsT] @ x_T          (N=256)
    k,v (n,j)  = [x_T chunk as lhsT]   @ W_qkv        (N=512)
    elu+1 on q_T and k as min(exp(.),1) + relu(.)     (Act/DVE/Pool)
    ctx (d,e)  = [k head as lhsT] @ v head            (N=128, 2 accums)
    attnT (e,n)= [ctx as lhsT]    @ q_T head          (N=256)
    out (n,c)  = [attnT chunk as lhsT] @ W_out + b    (N=512)

Software pipeline (PE emission order, steady state):
    kv_t0(w) ctx(w-1) kv_t1(w) attn(w-1) kv_rest(w) out(w-1) q(w+2)
(the interleaved kv tiles cover the ctx/attn PSUM->SBUF copy latencies)
with x_T tiles DMA'd two windows ahead and W_qkv streamed in column chunks
ordered so the pipeline never waits on weights. A short burst of dummy
matmuls at t=0 keeps the PE p-state ramp off the critical path, and the
last window drains through narrow bias-seeded tiles so the final
store chain after the last matmul is minimal.
"""

import numpy as np

P = 128
WS = 256          # window size
NW = 8            # windows per core
TOK = WS * NW     # 2048 tokens per core
D = 1024
J3 = 3 * D        # qkv width
H = 8
DH = 128
NCORES = 8
WARMUP = 30       # dummy PE matmuls to hold the p-state ramp

_CACHE = {}


def _build_nc(finalize=True, reps=1):
    import concourse.bass as bass
    import concourse.tile as tile
    from concourse import bacc, mybir
    from concourse.alu_op_type import AluOpType
    from contextlib import ExitStack

    f32 = mybir.dt.float32
    bf16 = mybir.dt.bfloat16
    AF = mybir.ActivationFunctionType

    nc = bacc.Bacc()
    xt_d = nc.declare_dram_parameter("xt", [NW, P, 8, WS], bf16, isOutput=False)
    wq_d = nc.declare_dram_parameter("w_qkv", [P, 8, J3], bf16, isOutput=False)
    wo_d = nc.declare_dram_parameter("w_out", [P, 8, D], bf16, isOutput=False)
    b_d = nc.declare_dram_parameter("b_out", [D], f32, isOutput=False)
    out_d = nc.declare_dram_parameter("out", [TOK, D], f32, isOutput=True)

    with ExitStack() as ctx:
        tc = ctx.enter_context(tile.TileContext(nc))
        consts = ctx.enter_context(tc.tile_pool(name="consts", bufs=1))
        xtp = ctx.enter_context(tc.tile_pool(name="xtp", bufs=4))
        qtp = ctx.enter_context(tc.tile_pool(name="qtp", bufs=4))
        work = ctx.enter_context(tc.tile_pool(name="work", bufs=2))
        tmps = ctx.enter_context(tc.tile_pool(name="tmps", bufs=2))
        obp = ctx.enter_context(tc.tile_pool(name="obp", bufs=3))
        ps_mm = ctx.enter_context(tc.tile_pool(name="ps_mm", bufs=6, space="PSUM"))
        ps_ctx = ctx.enter_context(tc.tile_pool(name="ps_ctx", bufs=2, space="PSUM"))

        # ---- resident constants, streamed in dependency order.  Chunk
        # boundaries are chosen so q(0) can start after ~3us and each later
        # pipeline stage's weights land before the PE reaches it. ----
        w_sb = consts.tile([P, 8, J3], bf16)     # W_qkv: part=d%128, mid=d//128
        wo_sb = consts.tile([P, 8, D], bf16)     # W_out: part=i%128, mid=i//128
        bias_sb = consts.tile([P, D], f32)       # b_out replicated on partitions
        dummy = consts.tile([P, P], bf16)        # warmup matmul operand

        xts = {}

        def load_xt(w, split=False):
            xt = xtp.tile([P, 8, WS], bf16, tag="xt", bufs=4)
            if split:
                nc.sync.dma_start(out=xt[:, 0:4, :], in_=xt_d[w, :, 0:4, :])
                nc.sync.dma_start(out=xt[:, 4:8, :], in_=xt_d[w, :, 4:8, :])
            else:
                nc.sync.dma_start(out=xt[:], in_=xt_d[w])
            xts[w] = xt

        def wq_chunk(a, b):
            nc.sync.dma_start(out=w_sb[:, :, a:b], in_=wq_d[:, :, a:b])

        wq_chunk(0, 256)
        load_xt(0, split=True)
        wq_chunk(256, 512)
        wq_chunk(512, 768)
        wq_chunk(768, 1024)
        load_xt(1)
        wq_chunk(1024, 1536)
        load_xt(2)
        wq_chunk(1536, 2048)
        wq_chunk(2048, 2560)
        wq_chunk(2560, 3072)
        for s in range(2):
            nc.sync.dma_start(out=wo_sb[:, :, s * 512:(s + 1) * 512],
                              in_=wo_d[:, :, s * 512:(s + 1) * 512])
        b_ap = b_d[:]
        bias_bcast = bass.AP(tensor=b_ap.tensor, offset=b_ap.offset,
                             ap=[[0, P]] + list(b_ap.ap))
        nc.sync.dma_start(out=bias_sb[:], in_=bias_bcast)

        # ---- PE warmup: dummy matmuls while the first weight chunks and
        # x tiles stream in, so real matmuls start at full clock ----
        nc.gpsimd.memset(dummy[:], 0.0)
        wu = ps_mm.tile([P, 512], f32, tag="mm", name="wu")
        for _ in range(WARMUP):
            nc.tensor.matmul(wu[:, :P], lhsT=dummy[:], rhs=dummy[:],
                             start=True, stop=True)

        state = {}

        def elu1(dst, src):
            # elu(x)+1 == min(exp(x),1) + relu(x); src is f32 PSUM,
            # dst/temps bf16 SBUF.  Pool can't read PSUM, so it gets the
            # SBUF-only combine step.
            n = src.shape[-1]
            e = tmps.tile([P, 512], bf16, tag="e", bufs=2)
            r = tmps.tile([P, 512], bf16, tag="r", bufs=2)
            nc.scalar.activation(e[:, :n], src, AF.Exp)
            nc.vector.tensor_scalar_max(r[:, :n], src, 0.0)
            nc.vector.scalar_tensor_tensor(
                out=dst, in0=e[:, :n], scalar=1.0, in1=r[:, :n],
                op0=AluOpType.min, op1=AluOpType.add)

        def stage_q(w):
            xt = xts[w]
            # q_T = elu(W_q^T x^T)+1 : (j,n); 2 heads per PSUM bank
            qt = qtp.tile([P, 8, WS], bf16, tag="qt", bufs=4)
            for t in range(4):
                qp = ps_mm.tile([P, 512], f32, tag="mm")
                for half in range(2):
                    jc = 2 * t + half
                    for dc in range(8):
                        nc.tensor.matmul(
                            qp[:, half * WS:(half + 1) * WS],
                            lhsT=w_sb[:, dc, jc * P:(jc + 1) * P],
                            rhs=xt[:, dc, :],
                            start=(dc == 0), stop=(dc == 7))
                elu1(qt[:, 2 * t:2 * t + 2, :], qp[:])
            state[(w, "qt")] = qt

        def stage_kv(w, tiles):
            xt = xts[w]
            if (w, "kn") not in state:
                kn = work.tile([P, 2, D], bf16, tag="kn", bufs=2, name="kn")
                vn = work.tile([P, 2, D], bf16, tag="vn", bufs=2, name="vn")
                state[(w, "kn")] = kn
                state[(w, "vn")] = vn
            kn = state[(w, "kn")]
            vn = state[(w, "vn")]
            for i, jc in tiles:   # jc: 4 x 512 across [k | v]
                kvp = ps_mm.tile([P, 512], f32, tag="mm")
                for dc in range(8):
                    nc.tensor.matmul(
                        kvp[:], lhsT=xt[:, dc, i * P:(i + 1) * P],
                        rhs=w_sb[:, dc, D + jc * 512:D + (jc + 1) * 512],
                        start=(dc == 0), stop=(dc == 7))
                if jc < 2:  # k columns: elu+1
                    elu1(kn[:, i, jc * 512:(jc + 1) * 512], kvp[:])
                else:       # v columns: plain copy
                    nc.scalar.copy(
                        out=vn[:, i, (jc - 2) * 512:(jc - 1) * 512],
                        in_=kvp[:])

        def stage_ctx(w):
            kn = state.pop((w, "kn"))
            vn = state.pop((w, "vn"))
            ctxs = work.tile([P, H, DH], bf16, tag="ctxs", bufs=2)
            for t in range(2):    # 4 heads per PSUM bank
                cp = ps_ctx.tile([P, 512], f32, tag="cps", bufs=2)
                for hh in range(4):
                    h = 4 * t + hh
                    for i in range(2):
                        nc.tensor.matmul(
                            cp[:, hh * DH:(hh + 1) * DH],
                            lhsT=kn[:, i, h * DH:(h + 1) * DH],
                            rhs=vn[:, i, h * DH:(h + 1) * DH],
                            start=(i == 0), stop=(i == 1))
                nc.scalar.copy(out=ctxs[:, 4 * t:4 * t + 4, :], in_=cp[:])
            state[(w, "ctxs")] = ctxs

        def stage_attn(w):
            qt = state.pop((w, "qt"))
            ctxs = state.pop((w, "ctxs"))
            at = work.tile([P, H, WS], bf16, tag="at", bufs=2)
            for t in range(4):    # 2 heads per PSUM bank
                ap_ = ps_mm.tile([P, 512], f32, tag="mm")
                for hh in range(2):
                    h = 2 * t + hh
                    nc.tensor.matmul(ap_[:, hh * WS:(hh + 1) * WS],
                                     lhsT=ctxs[:, h, :], rhs=qt[:, h, :],
                                     start=True, stop=True)
                nc.scalar.copy(out=at[:, 2 * t:2 * t + 2, :], in_=ap_[:])
            state[(w, "at")] = at

        def stage_out(w):
            at = state.pop((w, "at"))
            # Last tile of the last window: seed PSUM with the bias so the
            # post-matmul drain is two parallel half-copies + stores instead
            # of a serial bias-add + full store.
            last = (w == NW - 1)
            for i in range(2):
                for cc in range(2):
                    op = ps_mm.tile([P, 512], f32, tag="mm")
                    fin = last and i == 1 and cc == 1
                    if fin:
                        nc.vector.tensor_copy(op[:], bias_sb[:, 512:1024])
                    for hc in range(8):
                        nc.tensor.matmul(
                            op[:], lhsT=at[:, hc, i * P:(i + 1) * P],
                            rhs=wo_sb[:, hc, cc * 512:(cc + 1) * 512],
                            start=(hc == 0 and not fin), stop=(hc == 7),
                            skip_group_check=fin)
                    ob = obp.tile([P, 512], f32, tag="ob", bufs=3)
                    rows = out_d[w * WS + i * P: w * WS + (i + 1) * P, :]
                    if fin:
                        nc.vector.tensor_copy(ob[:, :256], op[:, :256])
                        nc.sync.dma_start(out=rows[:, 512:768],
                                          in_=ob[:, :256])
                        nc.scalar.copy(out=ob[:, 256:512], in_=op[:, 256:512])
                        nc.scalar.dma_start(out=rows[:, 768:1024],
                                            in_=ob[:, 256:512])
                    else:
                        nc.vector.tensor_add(ob[:], op[:],
                                             bias_sb[:, cc * 512:(cc + 1) * 512])
                        nc.sync.dma_start(
                            out=rows[:, cc * 512:(cc + 1) * 512], in_=ob[:])

        for _rep in range(reps):
            if _rep > 0:
                for w in range(3):
                    load_xt(w)
            ALL = [(i, jc) for i in range(2) for jc in range(4)]
            stage_q(0)
            stage_q(1)
            stage_q(2)
            stage_kv(0, ALL)
            for w in range(1, NW):
                if w + 2 < NW:
                    load_xt(w + 2)
                stage_kv(w, ALL[:1])
                stage_ctx(w - 1)
                stage_kv(w, ALL[1:2])
                stage_attn(w - 1)
                stage_kv(w, ALL[2:])
                if w == NW - 1:
                    stage_out(w - 1, [(0, 0), (0, 1), (1, 0)])
                else:
                    stage_out(w - 1)
                if w + 2 < NW:
                    stage_q(w + 2)
            stage_ctx(NW - 1)
            stage_out(NW - 2, [(1, 1)])
            stage_attn(NW - 1)
            stage_out(NW - 1)
    if finalize:
        nc.finalize()
    return nc


def _get_nc():
    if "nc" not in _CACHE:
        _CACHE["nc"] = _build_nc()
    return _CACHE["nc"]


def make_core_inputs(x, W_qkv, W_out, b_out):
    """Host-side shard + layout prep: returns per-core in_maps."""
    from concourse import mybir
    bf = mybir.dt.np(mybir.dt.bfloat16)

    x = np.asarray(x, dtype=np.float32)
    W_qkv = np.asarray(W_qkv, dtype=np.float32)
    W_out = np.asarray(W_out, dtype=np.float32)
    b_out = np.ascontiguousarray(np.asarray(b_out, dtype=np.float32))

    # W_qkv (D, 3D) -> (P, 8, 3D) with row d = c*128+p ; bf16
    wq = np.ascontiguousarray(
        W_qkv.astype(bf).reshape(8, P, J3).transpose(1, 0, 2))
    wo = np.ascontiguousarray(
        W_out.astype(bf).reshape(8, P, D).transpose(1, 0, 2))

    b, n, d = x.shape
    xf = x.reshape(b * n, d)
    in_maps = []
    for c in range(NCORES):
        xc = xf[c * TOK:(c + 1) * TOK].astype(bf)
        # (2048, 1024) -> [w, p, cc, n] = x[w*256+n, cc*128+p]
        xt = np.ascontiguousarray(
            xc.reshape(NW, WS, 8, P).transpose(0, 3, 2, 1))
        in_maps.append({"xt": xt, "w_qkv": wq, "w_out": wo, "b_out": b_out})
    return in_maps


def kernel(x, W_qkv, W_out, b_out):
    from concourse.bass_utils import run_bass_kernel_spmd

    nc = _get_nc()
    x = np.asarray(x, dtype=np.float32)
    b, n, d = x.shape
    in_maps = make_core_inputs(x, W_qkv, W_out, b_out)
    res = run_bass_kernel_spmd(nc, in_maps, list(range(NCORES)))
    out = np.concatenate([res.results[c]["out"] for c in range(NCORES)], axis=0)
    return out.reshape(b, n, d)


# revision 44
# speedup vs baseline: 1.2800x; 1.2800x over previous
"""Trainium2 Bass kernel for nn_LocalContextProcessor (local linear attention).

Computation (per 256-token window, fully independent):
    qkv = x @ W_qkv ; q,k,v split ; per head: q,k <- elu(.)+1
    ctx = k^T @ v ; attn = (q @ ctx) ; out = attn @ W_out + b_out

Sharding: data-parallel over the 64 windows (4 batch x 16 windows);
each of the 8 cores processes 8 consecutive windows (2048 tokens).
Weights are replicated to every core.

The three large GEMMs (qkv projection, output projection) run as fp8-e4m3
DoubleRow matmuls with a 3-pass hi/lo residual decomposition:
    x @ W  ~=  x8@W8 + dx8@W8 + x8@dW8        (f32 PSUM accumulate)
where x8/W8 are power-of-2-scaled fp8 quantizations and dx8/dW8 fp8
quantizations of the residuals (computed on the host; attn quantized
on-chip).  DoubleRow folds two 128-deep K-tiles per instruction at 0.5
cyc/row, so the 3 passes cost 0.75x of the bf16 equivalent while the
omitted dx@dW term and second-order quantization leave ~0.1% error --
below the plain-bf16 error.  The small ctx/attn matmuls stay bf16
(1 cyc/row at any width).  Every matmul runs at the PE's peak rate for
its dtype; per-core PE work is ~426k cycles (~177.5us at 2.4GHz).

Per-core dataflow (all matmuls contract over the partition dim):
    q_T (j,n)  = [W chunks as lhsT] @ x_T       (fp8 DR, 3 passes)
    k,v (n,j)  = [x_T chunks as lhsT] @ W       (fp8 DR, 3 passes)
    elu+1 on q_T and k as min(exp(.),1) + relu(.), rescaled by 1/512
    ctx (d,e)  = [k head as lhsT] @ v head      (bf16, N=128)
    attnT (e,n)= [ctx as lhsT]    @ q_T head    (bf16, N=256)
    attn quantized on-chip to at8 + dat8 (scale 2^-4)
    out (n,c)  = [attnT chunks as lhsT] @ W_out (fp8 DR, 3 passes) + b

Software pipeline (PE emission order, steady state):
    kv_t0(w) ctx(w-1) kv_t1(w) attn(w-1) kv_rest(w) out(w-1) q(w+2)
with x tiles DMA'd two windows ahead and the fp8 weight pairs streamed
in column chunks ordered to match consumption.  A burst of dummy
matmuls at t=0 keeps the PE p-state ramp off the critical path; the
last window drains through narrow bias-seeded tiles.
"""

import numpy as np

P = 128
WS = 256          # window size
NW = 8            # windows per core
TOK = WS * NW     # 2048 tokens per core
D = 1024
J3 = 3 * D        # qkv width
H = 8
DH = 128
NCORES = 8
WARMUP = 46       # dummy PE matmuls to hold the p-state ramp

SX = 8.0          # x pre-scale (host)
SW = 64.0         # weight pre-scale (host)
SA = 2.0 ** -7    # attn pre-scale (on-chip; attn absmax ~1.4e4, fp8 max 240)
RQKV = 1.0 / (SX * SW)   # PSUM rescale after qkv matmuls
ROUT = 1.0 / (SA * SW)   # PSUM rescale after out-proj matmuls

_CACHE = {}


def _build_nc(finalize=True, reps=1):
    import concourse.bass as bass
    import concourse.tile as tile
    from concourse import bacc, mybir
    from concourse.alu_op_type import AluOpType
    from contextlib import ExitStack

    f32 = mybir.dt.float32
    bf16 = mybir.dt.bfloat16
    fp8 = mybir.dt.float8e4
    AF = mybir.ActivationFunctionType
    DR = mybir.MatmulPerfMode.DoubleRow

    nc = bacc.Bacc()
    x8_d = nc.declare_dram_parameter("x8", [NW, P, 8, WS], fp8, isOutput=False)
    dx8_d = nc.declare_dram_parameter("dx8", [NW, P, 8, WS], fp8, isOutput=False)
    w8_d = nc.declare_dram_parameter("w8", [P, 8, J3], fp8, isOutput=False)
    dw8_d = nc.declare_dram_parameter("dw8", [P, 8, J3], fp8, isOutput=False)
    wo8_d = nc.declare_dram_parameter("wo8", [P, 8, D], fp8, isOutput=False)
    dwo8_d = nc.declare_dram_parameter("dwo8", [P, 8, D], fp8, isOutput=False)
    b_d = nc.declare_dram_parameter("b_out", [D], f32, isOutput=False)
    b4_d = nc.declare_dram_parameter("b_seed", [D], f32, isOutput=False)
    out_d = nc.declare_dram_parameter("out", [TOK, D], f32, isOutput=True)

    with ExitStack() as ctx:
        tc = ctx.enter_context(tile.TileContext(nc))
        consts = ctx.enter_context(tc.tile_pool(name="consts", bufs=1))
        xtp = ctx.enter_context(tc.tile_pool(name="xtp", bufs=4))
        qtp = ctx.enter_context(tc.tile_pool(name="qtp", bufs=4))
        work = ctx.enter_context(tc.tile_pool(name="work", bufs=2))
        tmps = ctx.enter_context(tc.tile_pool(name="tmps", bufs=2))
        obp = ctx.enter_context(tc.tile_pool(name="obp", bufs=3))
        ps_mm = ctx.enter_context(tc.tile_pool(name="ps_mm", bufs=6, space="PSUM"))
        ps_ctx = ctx.enter_context(tc.tile_pool(name="ps_ctx", bufs=2, space="PSUM"))

        w8_sb = consts.tile([P, 8, J3], fp8)
        dw8_sb = consts.tile([P, 8, J3], fp8)
        wo8_sb = consts.tile([P, 8, D], fp8)
        dwo8_sb = consts.tile([P, 8, D], fp8)
        bias_sb = consts.tile([P, D], f32)
        bias4_sb = consts.tile([P, D], f32)
        dummy = consts.tile([P, P], bf16)

        xts = {}

        def load_xt(w):
            x8 = xtp.tile([P, 8, WS], fp8, tag="x8", bufs=4, name="x8")
            dx8 = xtp.tile([P, 8, WS], fp8, tag="dx8", bufs=4, name="dx8")
            nc.sync.dma_start(out=x8[:], in_=x8_d[w])
            nc.sync.dma_start(out=dx8[:], in_=dx8_d[w])
            xts[w] = (x8, dx8)

        def chunk(sb, d, a, b):
            nc.sync.dma_start(out=sb[:, :, a:b], in_=d[:, :, a:b])

        # weight/x streaming in consumption order (q cols first, each w8
        # chunk followed by its dw8 chunk, kv cols later, W_out last)
        chunk(w8_sb, w8_d, 0, 512)
        load_xt(0)
        chunk(dw8_sb, dw8_d, 0, 512)
        load_xt(1)
        load_xt(2)
        chunk(w8_sb, w8_d, 512, 1024)
        chunk(dw8_sb, dw8_d, 512, 1024)
        for s in range(4):
            chunk(w8_sb, w8_d, 1024 + s * 512, 1536 + s * 512)
            chunk(dw8_sb, dw8_d, 1024 + s * 512, 1536 + s * 512)
        for s in range(2):
            chunk(wo8_sb, wo8_d, s * 512, (s + 1) * 512)
            chunk(dwo8_sb, dwo8_d, s * 512, (s + 1) * 512)
        for bd, bsb in ((b_d, bias_sb), (b4_d, bias4_sb)):
            b_ap = bd[:]
            bcast = bass.AP(tensor=b_ap.tensor, offset=b_ap.offset,
                            ap=[[0, P]] + list(b_ap.ap))
            nc.sync.dma_start(out=bsb[:], in_=bcast)

        # ---- PE warmup ----
        nc.gpsimd.memset(dummy[:], 0.0)
        wu = ps_mm.tile([P, 512], f32, tag="mm", name="wu")
        for _ in range(WARMUP):
            nc.tensor.matmul(wu[:, :P], lhsT=dummy[:], rhs=dummy[:],
                             start=True, stop=True)

        state = {}

        def elu1(dst, src):
            # elu(x)+1 == min(exp(x),1) + relu(x), with the 1/(SX*SW)
            # de-scale of the fp8 matmul result fused into exp and relu.
            n = src.shape[-1]
            e = tmps.tile([P, 512], bf16, tag="e", bufs=2)
            r = tmps.tile([P, 512], bf16, tag="r", bufs=2)
            nc.scalar.activation(e[:, :n], src, AF.Exp, scale=RQKV)
            nc.vector.tensor_scalar(r[:, :n], src, 0.0, RQKV,
                                    op0=AluOpType.max, op1=AluOpType.mult)
            nc.vector.scalar_tensor_tensor(
                out=dst, in0=e[:, :n], scalar=1.0, in1=r[:, :n],
                op0=AluOpType.min, op1=AluOpType.add)

        def qkv_passes(x8, dx8):
            return ((x8, w8_sb), (dx8, w8_sb), (x8, dw8_sb))

        def stage_q(w, tiles=(0, 1, 2, 3), pnos=(0, 1, 2)):
            # q_T (j,n): stationary = W columns, moving = x_T; 2 heads per
            # PSUM bank, 3 fp8 passes x 4 DoubleRow K-pair steps each.
            # Passes of a tile may be emitted across separate calls (the
            # prologue runs hi-passes for three windows before the residual
            # operands have streamed in); PSUM group state is kept in
            # `state` and elu fires when a tile's 12 steps complete.
            x8, dx8 = xts[w]
            if (w, "qt") not in state:
                qt = qtp.tile([P, 8, WS], bf16, tag="qt", bufs=4, name="qt")
                state[(w, "qt")] = qt
            qt = state[(w, "qt")]
            qps = state.setdefault((w, "qps"), {})
            qni = state.setdefault((w, "qni"), {})
            passes = qkv_passes(x8, dx8)
            for t in tiles:
                if t not in qps:
                    qps[t] = ps_mm.tile([P, 512], f32, tag="mm", name="qp")
                for half in range(2):
                    jc = 2 * t + half
                    for pno in pnos:
                        X_, W_ = passes[pno]
                        for dp in range(4):
                            n = qni.get((t, half), 0)
                            nc.tensor.matmul(
                                qps[t][:, half * WS:(half + 1) * WS],
                                lhsT=W_[:, 2 * dp:2 * dp + 2,
                                        jc * P:(jc + 1) * P],
                                rhs=X_[:, 2 * dp:2 * dp + 2, :],
                                start=(n == 0), stop=(n == 11),
                                perf_mode=DR, skip_group_check=True)
                            qni[(t, half)] = n + 1
                if qni.get((t, 0), 0) == 12 and qni.get((t, 1), 0) == 12:
                    elu1(qt[:, 2 * t:2 * t + 2, :], qps.pop(t)[:])

        def stage_kv(w, tiles):
            x8, dx8 = xts[w]
            if (w, "kn") not in state:
                kn = work.tile([P, 2, D], bf16, tag="kn", bufs=2, name="kn")
                vn = work.tile([P, 2, D], bf16, tag="vn", bufs=2, name="vn")
                state[(w, "kn")] = kn
                state[(w, "vn")] = vn
            kn = state[(w, "kn")]
            vn = state[(w, "vn")]
            for i, jc in tiles:   # jc: 4 x 512 across [k | v]
                kvp = ps_mm.tile([P, 512], f32, tag="mm")
                ni = 0
                for X_, W_ in qkv_passes(x8, dx8):
                    for dp in range(4):
                        nc.tensor.matmul(
                            kvp[:],
                            lhsT=X_[:, 2 * dp:2 * dp + 2, i * P:(i + 1) * P],
                            rhs=W_[:, 2 * dp:2 * dp + 2,
                                   D + jc * 512:D + (jc + 1) * 512],
                            start=(ni == 0), stop=(ni == 11),
                            perf_mode=DR)
                        ni += 1
                if jc < 2:  # k columns: elu+1 (rescaled)
                    elu1(kn[:, i, jc * 512:(jc + 1) * 512], kvp[:])
                else:       # v columns: rescaled copy to bf16
                    nc.scalar.activation(
                        vn[:, i, (jc - 2) * 512:(jc - 1) * 512], kvp[:],
                        AF.Copy, scale=RQKV)

        def stage_ctx(w):
            kn = state.pop((w, "kn"))
            vn = state.pop((w, "vn"))
            ctxs = work.tile([P, H, DH], bf16, tag="ctxs", bufs=2)
            for t in range(2):    # 4 heads per PSUM bank
                cp = ps_ctx.tile([P, 512], f32, tag="cps", bufs=2)
                for hh in range(4):
                    h = 4 * t + hh
                    for i in range(2):
                        nc.tensor.matmul(
                            cp[:, hh * DH:(hh + 1) * DH],
                            lhsT=kn[:, i, h * DH:(h + 1) * DH],
                            rhs=vn[:, i, h * DH:(h + 1) * DH],
                            start=(i == 0), stop=(i == 1))
                nc.scalar.copy(out=ctxs[:, 4 * t:4 * t + 4, :], in_=cp[:])
            state[(w, "ctxs")] = ctxs

        def stage_attn(w):
            qt = state.pop((w, "qt"))
            ctxs = state.pop((w, "ctxs"))
            # attn stays bf16; its result is quantized on-chip to
            # at8 + dat8 (scale SA) for the fp8 out-projection.
            at8 = work.tile([P, H, WS], fp8, tag="at8", bufs=2, name="at8")
            dat8 = work.tile([P, H, WS], fp8, tag="dat8", bufs=2, name="dat8")
            from concourse.alu_op_type import AluOpType
            for t in range(4):    # 2 heads per PSUM bank
                ap_ = ps_mm.tile([P, 512], f32, tag="mm")
                for hh in range(2):
                    h = 2 * t + hh
                    nc.tensor.matmul(ap_[:, hh * WS:(hh + 1) * WS],
                                     lhsT=ctxs[:, h, :], rhs=qt[:, h, :],
                                     start=True, stop=True)
                a8s = at8[:, 2 * t:2 * t + 2, :]
                nc.scalar.activation(a8s, ap_[:], AF.Copy, scale=SA)
                nc.vector.scalar_tensor_tensor(
                    out=dat8[:, 2 * t:2 * t + 2, :], in0=ap_[:], scalar=SA,
                    in1=a8s, op0=AluOpType.mult, op1=AluOpType.subtract)
            state[(w, "at8")] = at8
            state[(w, "dat8")] = dat8

        def out_passes(at8, dat8):
            return ((at8, wo8_sb), (dat8, wo8_sb), (at8, dwo8_sb))

        def stage_out(w, tiles=None):
            from concourse.alu_op_type import AluOpType
            at8 = state[(w, "at8")]
            dat8 = state[(w, "dat8")]
            last = (w == NW - 1)
            for i, cc in (tiles if tiles is not None
                          else [(i, cc) for i in range(2) for cc in range(2)]):
                    fin = last and i == 1 and cc == 1
                    rows = out_d[w * WS + i * P: w * WS + (i + 1) * P, :]
                    if fin:
                        # bias-seeded PSUM (b*SA*SW compensates the ROUT
                        # de-scale); drain = narrow parallel scaled copies.
                        ob = obp.tile([P, 512], f32, tag="ob", bufs=3)
                        ob2 = obp.tile([P, 256], f32, tag="ob2", bufs=1,
                                       name="ob2")
                        for sub, c0, c1 in ((0, 0, 256), (1, 256, 384),
                                            (2, 384, 512)):
                            op = ps_mm.tile([P, 512], f32, tag="mm")
                            w_ = c1 - c0
                            nc.vector.tensor_copy(
                                op[:, :w_], bias4_sb[:, 512 + c0:512 + c1])
                            ni = 0
                            for A_, W_ in out_passes(at8, dat8):
                                for hp in range(4):
                                    nc.tensor.matmul(
                                        op[:, :w_],
                                        lhsT=A_[:, 2 * hp:2 * hp + 2,
                                                i * P:(i + 1) * P],
                                        rhs=W_[:, 2 * hp:2 * hp + 2,
                                               512 + c0:512 + c1],
                                        start=False, stop=(ni == 11),
                                        perf_mode=DR, skip_group_check=True)
                                    ni += 1
                            if sub == 0:
                                nc.vector.tensor_scalar_mul(
                                    ob[:, :256], op[:, :256], ROUT)
                                nc.sync.dma_start(out=rows[:, 512:768],
                                                  in_=ob[:, :256])
                            elif sub == 1:
                                nc.vector.tensor_scalar_mul(
                                    ob2[:, :128], op[:, :128], ROUT)
                                nc.sync.dma_start(out=rows[:, 768:896],
                                                  in_=ob2[:, :128])
                            else:
                                nc.scalar.activation(
                                    ob2[:, 128:256], op[:, :128],
                                    AF.Copy, scale=ROUT)
                                nc.scalar.dma_start(out=rows[:, 896:1024],
                                                    in_=ob2[:, 128:256])
                        continue
                    op = ps_mm.tile([P, 512], f32, tag="mm")
                    ni = 0
                    for A_, W_ in out_passes(at8, dat8):
                        for hp in range(4):
                            nc.tensor.matmul(
                                op[:],
                                lhsT=A_[:, 2 * hp:2 * hp + 2, i * P:(i + 1) * P],
                                rhs=W_[:, 2 * hp:2 * hp + 2,
                                       cc * 512:(cc + 1) * 512],
                                start=(ni == 0), stop=(ni == 11),
                                perf_mode=DR)
                            ni += 1
                    ob = obp.tile([P, 512], f32, tag="ob", bufs=3)
                    # ob = op * ROUT + bias
                    nc.vector.scalar_tensor_tensor(
                        out=ob[:], in0=op[:], scalar=ROUT,
                        in1=bias_sb[:, cc * 512:(cc + 1) * 512],
                        op0=AluOpType.mult, op1=AluOpType.add)
                    if last and i == 0:
                        nc.scalar.dma_start(
                            out=rows[:, cc * 512:(cc + 1) * 512], in_=ob[:])
                    else:
                        nc.sync.dma_start(
                            out=rows[:, cc * 512:(cc + 1) * 512], in_=ob[:])

        for _rep in range(reps):
            if _rep > 0:
                for w in range(3):
                    load_xt(w)
            ALL = [(i, jc) for i in range(2) for jc in range(4)]
            for tt in (0, 2):
                for w in range(3):
                    stage_q(w, (tt, tt + 1))
            stage_kv(0, [(i, jc) for jc in range(4) for i in range(2)])
            for w in range(1, NW):
                if w + 2 < NW:
                    load_xt(w + 2)
                stage_kv(w, ALL[:1])
                stage_ctx(w - 1)
                stage_kv(w, ALL[1:2])
                stage_attn(w - 1)
                stage_kv(w, ALL[2:])
                if w == NW - 1:
                    stage_out(w - 1, [(0, 0), (0, 1), (1, 0)])
                else:
                    stage_out(w - 1)
                if w + 2 < NW:
                    stage_q(w + 2)
            stage_ctx(NW - 1)
            stage_out(NW - 2, [(1, 1)])
            stage_attn(NW - 1)
            stage_out(NW - 1)
    if finalize:
        nc.finalize()
    return nc


def _get_nc():
    if "nc" not in _CACHE:
        _CACHE["nc"] = _build_nc()
    return _CACHE["nc"]


def make_core_inputs(x, W_qkv, W_out, b_out):
    """Host-side shard + fp8 hi/lo quantization + layout prep."""
    from concourse import mybir
    f8 = mybir.dt.np(mybir.dt.float8e4)

    x = np.asarray(x, dtype=np.float32)
    W_qkv = np.asarray(W_qkv, dtype=np.float32)
    W_out = np.asarray(W_out, dtype=np.float32)
    b_out = np.ascontiguousarray(np.asarray(b_out, dtype=np.float32))
    b_seed = np.ascontiguousarray((SA * SW) * b_out)

    def hilo(a):
        hi = a.astype(f8)
        lo = (a - hi.astype(np.float32)).astype(f8)
        return hi, lo

    # W_qkv (D, 3D) -> (P, 8, 3D) with row d = c*128+p ; scaled fp8 pair
    wq_s = (W_qkv * SW).reshape(8, P, J3).transpose(1, 0, 2)
    w8, dw8 = hilo(np.ascontiguousarray(wq_s))
    wo_s = (W_out * SW).reshape(8, P, D).transpose(1, 0, 2)
    wo8, dwo8 = hilo(np.ascontiguousarray(wo_s))

    b, n, d = x.shape
    xf = x.reshape(b * n, d)
    in_maps = []
    for c in range(NCORES):
        # (2048, 1024) -> [w, p, cc, n] = x[w*256+n, cc*128+p] ; scaled
        xc = (xf[c * TOK:(c + 1) * TOK] * SX)
        xt = np.ascontiguousarray(
            xc.reshape(NW, WS, 8, P).transpose(0, 3, 2, 1))
        x8, dx8 = hilo(xt)
        in_maps.append({"x8": x8, "dx8": dx8, "w8": w8, "dw8": dw8,
                        "wo8": wo8, "dwo8": dwo8,
                        "b_out": b_out, "b_seed": b_seed})
    return in_maps


def kernel(x, W_qkv, W_out, b_out):
    from concourse.bass_utils import run_bass_kernel_spmd

    nc = _get_nc()
    x = np.asarray(x, dtype=np.float32)
    b, n, d = x.shape
    in_maps = make_core_inputs(x, W_qkv, W_out, b_out)
    res = run_bass_kernel_spmd(nc, in_maps, list(range(NCORES)))
    out = np.concatenate([res.results[c]["out"] for c in range(NCORES)], axis=0)
    return out.reshape(b, n, d)
